# revision 19
# baseline (speedup 1.0000x reference)
"""2D Haar DWT (LL subband) on 8 Trainium2 NeuronCores.

Reference computes LL = M0 @ x @ M1 per (n, c) image, where M0/M1 are the
Haar analysis low-pass matrices: every output element is the 2x2 box sum of
the input scaled by (1/sqrt(2))^2.  That makes the kernel a pure streaming
2x2-pool: memory-bound, no matmul needed.

Sharding: data-parallel over N (8 images of (32, 512, 512) -> one per core),
no communication.  Per core, each (512, 512) channel is loaded as one
contiguous [128, 2048] SBUF tile (4 consecutive rows per partition), reduced
with one contiguous 3D-AP row-pair add + one stride-2 column-pair add on DVE,
scaled by c^2 on ACT, and stored as one contiguous [128, 512] tile (2
output rows per partition).

Raw Bass (no Tile): the SP sequencer issues input DMAs, ACT issues the
output DMAs right after its scale op.  All cross-engine deps are standalone
wait_ge instructions on the consuming sequencer, so no DMA descriptor ever
carries more than its completion increment (walrus DIRECT2D allows only one
embedded sync wait).
"""

import time
from contextlib import ExitStack

import numpy as np

import concourse.bass as bass
import concourse.mybir as mybir
from concourse.bass_utils import run_bass_kernel_spmd

N, C, H, W = 8, 32, 512, 512
N_CORES = 8
# Match the reference's effective multiplier fl(c)*fl(c), c = f32(1/sqrt(2)).
_C = np.float32(1.0) / np.sqrt(np.float32(2.0))
C2 = float(np.float32(_C * _C))

_F32 = mybir.dt.float32


def build_nc(B: int = 8, n_img: int = C, h: int = H, w: int = W) -> bass.Bass:
    """B = pipeline depth (SBUF slots per stage); B*2MiB of SBUF used."""
    C_, H_, W = n_img, h, w  # noqa: N806 - shadow module constants for the body
    R = H_ // 256  # row-pair groups per partition (tile holds 2R rows/partition)
    assert H_ % 256 == 0 and R >= 1
    nc = bass.Bass()
    x = nc.dram_tensor("x", [C_, H_, W], _F32, kind="ExternalInput")
    y = nc.dram_tensor("y", [C_, H_ // 2, W // 2], _F32, kind="ExternalOutput")

    with ExitStack() as ctx:
        t = [
            ctx.enter_context(nc.sbuf_tensor(f"t{i}", [128, 2 * R * W], _F32))
            for i in range(B)
        ]
        v = [
            ctx.enter_context(nc.sbuf_tensor(f"v{i}", [128, R * W], _F32))
            for i in range(B)
        ]
        o = [
            ctx.enter_context(nc.sbuf_tensor(f"o{i}", [128, R * W // 2], _F32))
            for i in range(B)
        ]
        s = [
            ctx.enter_context(nc.sbuf_tensor(f"s{i}", [128, R * W // 2], _F32))
            for i in range(B)
        ]

        # DMA completions across dynamic queues are unordered, so a single
        # counting semaphore cannot identify WHICH transfer finished: use one
        # semaphore per buffer slot (only that slot's DMA bumps it).
        dma_in = [nc.alloc_semaphore(f"dma_in{i}") for i in range(B)]
        dma_out = [nc.alloc_semaphore(f"dma_out{i}") for i in range(B)]
        dve_sem = nc.alloc_semaphore("dve_sem")
        act_sem = nc.alloc_semaphore("act_sem")

        with nc.Block() as block:

            @block.sync
            def _(sync):
                for c in range(C_):
                    if c >= B:
                        # WAR: vadd(c-B) must be done reading t-slot; this also
                        # transitively orders WAW vs in-DMA(c-B).
                        sync.wait_ge(dve_sem, 2 * (c - B) + 1)
                    sync.dma_start(
                        t[c % B][:], x[c].rearrange("(p r) w -> p (r w)", p=128)
                    ).then_inc(dma_in[c % B], 16)
                for i in range(B):
                    rounds = C_ // B + (1 if i < C_ % B else 0)
                    if rounds:
                        sync.wait_ge(dma_out[i], 16 * rounds)

            @block.vector
            def _(vector):
                for c in range(C_):
                    vector.wait_ge(dma_in[c % B], 16 * (c // B + 1))
                    if c >= B:
                        # WAR: hadd(c-B) must be done reading v-slot
                        vector.wait_ge(dve_sem, 2 * (c - B) + 2)
                    # vertical pair sums over each adjacent row pair
                    tt = t[c % B][:].rearrange("p (r q w) -> p r q w", r=R, q=2)
                    vector.tensor_add(
                        v[c % B][:].rearrange("p (r w) -> p r w", r=R),
                        tt[:, :, 0, :],
                        tt[:, :, 1, :],
                    ).then_inc(dve_sem)
                    # RAW: engines are pipelined, same-engine back-to-back needs sync
                    vector.wait_ge(dve_sem, 2 * c + 1)
                    if c >= B:
                        # WAR: scale(c-B) must be done reading o-slot
                        vector.wait_ge(act_sem, c - B + 1)
                    vv = v[c % B][:].rearrange("p (w two) -> p two w", two=2)
                    vector.tensor_add(o[c % B][:], vv[:, 0, :], vv[:, 1, :]).then_inc(
                        dve_sem
                    )

            @block.scalar
            def _(scalar):
                for c in range(C_):
                    scalar.wait_ge(dve_sem, 2 * (c + 1))
                    if c >= B:
                        # WAR: out-DMA(c-B) must be done reading s-slot
                        scalar.wait_ge(dma_out[c % B], 16 * (c // B))
                    scalar.mul(s[c % B][:], o[c % B][:], C2).then_inc(act_sem)
                    # RAW on s: embedded wait (a DIRECT2D DMA allows exactly one)
                    scalar.dma_start(
                        y[c].rearrange("(p r) w -> p (r w)", p=128), s[c % B][:]
                    ).then_inc(dma_out[c % B], 16)._wait_ge(act_sem, c + 1)

    return nc


_NC_CACHE: bass.Bass | None = None

# The kernel program processes "units" of 1024 contiguous rows (2 channels at
# a time): one 2 MiB input DMA with 16 KiB/partition descriptors and one
# 512 KiB output DMA with 4 KiB/partition descriptors.  Fewer, larger DMAs
# than per-channel units at the same modeled time (descriptor-count halved).
_UNITS, _UH, _B = C // 2, 2 * H, 6


def run(x: np.ndarray, **spmd_kwargs):
    """x: (8, 32, 512, 512) f32 -> BassKernelResults over the 8 cores."""
    global _NC_CACHE
    if _NC_CACHE is None:
        _NC_CACHE = build_nc(B=_B, n_img=_UNITS, h=_UH)
    in_maps = [
        {"x": np.ascontiguousarray(x[n], dtype=np.float32).reshape(_UNITS, _UH, W)}
        for n in range(N_CORES)
    ]
    return run_bass_kernel_spmd(_NC_CACHE, in_maps, list(range(N_CORES)), **spmd_kwargs)


def kernel(**inputs: np.ndarray) -> np.ndarray:
    global _NC_CACHE
    x = np.asarray(inputs["input"], dtype=np.float32)
    last_err = None
    for attempt in range(3):
        try:
            res = run(x)
            return _out_full(res)
        except Exception as e:  # transient NRT/axon exec-unit flakes: rebuild + retry
            last_err = e
            _NC_CACHE = None
            time.sleep(5.0)
    raise last_err


def _out_full(res) -> np.ndarray:
    return np.stack(
        [res.results[i]["y"].reshape(C, H // 2, W // 2) for i in range(N_CORES)], axis=0
    )
